# revision 1
# baseline (speedup 1.0000x reference)
# Trainium2 Bass kernel for a causal multi-head attention block.
#
# Reference computation (fp32):
#   qkv = x @ w_attn + b_attn ; split into q,k,v heads (N=16, H=64)
#   scores = q @ k^T / sqrt(H), causal mask, softmax over keys
#   out = (weights @ v) reshaped, then out @ w_proj + b_proj
#
# Sharding: 8 cores = 2 batches x 4 head-groups (4 heads each).
#   - batch data-parallel, heads tensor-parallel (c_attn columns / c_proj rows)
#   - each core emits a partial [T, D] projection output; host sums the 4
#     head-group partials per batch and adds b_proj (the gather step).
#
# On-device layout trick: scores are computed TRANSPOSED (S^T[s,t]) so that
# exp(S^T) tiles are directly usable as the stationary operand of the
# weights@V matmul (contraction over s = partition dim), eliminating all
# softmax-weight transposes.  Row sums come free via a ones-column in V.

import math

import numpy as np

B, T, D = 2, 2048, 1024
NHEAD, H = 16, 64
HPC = 4            # heads per core
CD = HPC * H       # 256 head-dim columns per core
N_CORES = 8
P = 128            # partitions
TT = T // P        # 16 t-tiles of 128
TB = T // 512      # 4 t-blocks of 512
KD = D // P        # 8 contraction tiles over D

_CACHE = {}


def _build_module(mm_dt_name: str):
    import contextlib

    import concourse.bass as bass  # noqa: F401
    import concourse.mybir as mybir
    import concourse.tile as tile
    from concourse import bacc

    f32 = mybir.dt.float32
    mdt = getattr(mybir.dt, mm_dt_name)
    # dtype for the softmax-weight @ V matmul operands: bf16 runs at
    # 1 cycle/row for any free-dim (fp32r pays 4x below N=256) and gets
    # fast weight loads on HW.  The row-sum is computed from the same
    # bf16 weights, so normalization cancels most of the rounding error.
    avdt = mybir.dt.bfloat16 if mm_dt_name == "float32r" else mdt

    nc = bacc.Bacc("TRN2", target_bir_lowering=False, debug=False)

    x_d = nc.dram_tensor("x", [T, D], mdt, kind="ExternalInput").ap()
    wqk_d = nc.dram_tensor("wqk", [D, 2 * CD], mdt, kind="ExternalInput").ap()
    bqk_d = nc.dram_tensor("bqk", [P, 4], f32, kind="ExternalInput").ap()
    wv_d = nc.dram_tensor("wv", [D, CD], mdt, kind="ExternalInput").ap()
    bv_d = nc.dram_tensor("bv", [P, CD], mdt, kind="ExternalInput").ap()
    wp_d = nc.dram_tensor("wp", [CD, D], mdt, kind="ExternalInput").ap()
    ident_d = nc.dram_tensor("ident", [P, P], mdt, kind="ExternalInput").ap()
    mask_d = nc.dram_tensor("mask", [P, P], avdt, kind="ExternalInput").ap()
    onescol_d = nc.dram_tensor("onescol", [P, 2 * HPC], avdt, kind="ExternalInput").ap()
    y_d = nc.dram_tensor("y", [T, D], f32, kind="ExternalOutput").ap()

    with tile.TileContext(nc) as tc, contextlib.ExitStack() as ctx:
        const_p = ctx.enter_context(tc.tile_pool(name="const", bufs=1))
        w_p = ctx.enter_context(tc.tile_pool(name="weights", bufs=1))
        x_p = ctx.enter_context(tc.tile_pool(name="xin", bufs=8))
        xt_p = ctx.enter_context(tc.tile_pool(name="xt", bufs=2))
        qkt_p = ctx.enter_context(tc.tile_pool(name="qkt", bufs=1))
        v_p = ctx.enter_context(tc.tile_pool(name="vbuf", bufs=1))
        e_p = ctx.enter_context(tc.tile_pool(name="epool", bufs=16))
        attn_p = ctx.enter_context(tc.tile_pool(name="attn", bufs=1))
        small_p = ctx.enter_context(tc.tile_pool(name="small", bufs=8))
        # single PSUM pool, 8 banks total:
        #   wps   [128,512]x2  (phase1 transposes/qkT/V + proj)      2 banks
        #   sp    [128,1024]x2 (scores)                              4 banks
        #   accp* [128,264]x2  (AV accumulators, 4 groups per bank)  2 banks
        psp = ctx.enter_context(tc.tile_pool(name="psp", bufs=2, space="PSUM"))

        # ---- loads, ordered by when phase 1 needs them, spread across
        # the three DMA-capable engines' queues ----
        x_dma_engines = [nc.sync, nc.scalar]
        rr = [0]

        def load(tile_ap, dram_ap):
            x_dma_engines[rr[0] % 2].dma_start(tile_ap, dram_ap)
            rr[0] += 1

        ident = const_p.tile([P, P], mdt, name="ident_sb")
        nc.sync.dma_start(ident, ident_d)

        x_sb = {}
        for g in range(4):
            xt_in = x_p.tile([P, D], mdt, name="x_sb", tag="x_sb")
            # keep the phase-gating first loads on the fast HWDGE queues
            # (SWDGE dispatch on gpsimd adds ~microseconds of latency)
            (nc.sync if g % 2 == 0 else nc.scalar).dma_start(
                xt_in, x_d[P * g : P * (g + 1), :]
            )
            x_sb[g] = xt_in

        bqk = const_p.tile([P, 4], f32, name="bqk_sb")
        nc.sync.dma_start(bqk, bqk_d)
        wqk_sb = []
        for k in range(KD):
            t = w_p.tile([P, 2 * CD], mdt, name=f"wqk{k}", tag=f"wqk{k}")
            nc.sync.dma_start(t, wqk_d[P * k : P * (k + 1), :])
            wqk_sb.append(t)
        wv_sb = []
        for k in range(KD):
            t = w_p.tile([P, CD], mdt, name=f"wv{k}", tag=f"wv{k}")
            nc.sync.dma_start(t, wv_d[P * k : P * (k + 1), :])
            wv_sb.append(t)
        bv = const_p.tile([P, CD], mdt, name="bv_sb")
        nc.sync.dma_start(bv, bv_d)
        onescol = const_p.tile([P, 2 * HPC], avdt, name="onescol_sb")
        nc.sync.dma_start(onescol, onescol_d)
        mask = const_p.tile([P, P], avdt, name="mask_sb")
        nc.sync.dma_start(mask, mask_d)
        for g in range(4, 8):
            xt_in = x_p.tile([P, D], mdt, name="x_sb", tag="x_sb")
            load(xt_in, x_d[P * g : P * (g + 1), :])
            x_sb[g] = xt_in
        wp_sb = []
        for c in range(CD // P):
            t = w_p.tile([P, D], mdt, name=f"wp{c}", tag=f"wp{c}")
            nc.sync.dma_start(t, wp_d[P * c : P * (c + 1), :])
            wp_sb.append(t)

        # persistent activation buffers
        qkt_sb = {}
        for m in range(4):
            for j in range(TB):
                qkt_sb[(m, j)] = qkt_p.tile(
                    [P, 512], mdt, name=f"qkt{m}_{j}", tag=f"qkt{m}_{j}"
                )
        v_sb = []
        for i in range(TT):
            v_sb.append(
                v_p.tile([P, HPC * (H + 2)], avdt, name=f"v{i}", tag=f"v{i}")
            )
        # one tile per (t-block, head-pair): the projection's c-th
        # transpose batch depends only on head-pair c's normalizes
        attn_t = {
            (tb, c): attn_p.tile([P, 4 * P], mdt, name=f"attn{tb}_{c}",
                                 tag=f"attn{tb}_{c}")
            for tb in range(TB)
            for c in range(2)
        }
        G = H + 2

        def phase1_block(j):
            """x^T transposes + qk^T + V for 512-wide t-block j."""
            for ti in range(4):
                g = 4 * j + ti
                if g in x_sb:
                    continue
                xt_in = x_p.tile([P, D], mdt, name="x_sb", tag="x_sb")
                load(xt_in, x_d[P * g : P * (g + 1), :])
                x_sb[g] = xt_in
            xt_blk = []
            for k in range(KD):
                pt = psp.tile([P, 512], mdt, name="xtp", tag="wps")
                for ti in range(4):
                    nc.tensor.transpose(
                        pt[:, P * ti : P * (ti + 1)],
                        x_sb[4 * j + ti][:, P * k : P * (k + 1)],
                        ident,
                    )
                xt = xt_p.tile([P, 512], mdt, name=f"xt{k}", tag=f"xt{k}")
                nc.vector.tensor_copy(xt, pt)
                xt_blk.append(xt)

            for m in (0, 2, 1, 3):  # head-pair 0 needs m0+m2: finish first
                ps = psp.tile([P, 512], f32, name="qkp", tag="wps")
                for k in range(KD):
                    nc.tensor.matmul(
                        ps,
                        wqk_sb[k][:, P * m : P * (m + 1)],
                        xt_blk[k],
                        start=(k == 0),
                        stop=(k == KD - 1),
                    )
                # psum->sbuf drain with the per-partition qk bias folded in
                nc.vector.tensor_scalar_add(
                    qkt_sb[(m, j)], ps, bqk[:, m : m + 1]
                )

            for ti in range(4):
                g = 4 * j + ti
                ps = psp.tile([P, CD], f32, name="vp", tag="wps")
                for k in range(KD):
                    nc.tensor.matmul(
                        ps,
                        xt_blk[k][:, P * ti : P * (ti + 1)],
                        wv_sb[k],
                        start=(k == 0),
                        stop=(k == KD - 1),
                    )
                # psum->sbuf drain with the (partition-broadcast) V bias
                vg = v_sb[g].rearrange("p (g c) -> p g c", g=HPC)
                nc.vector.tensor_add(
                    vg[:, :, 0:H],
                    ps.rearrange("p (g c) -> p g c", g=HPC),
                    bv.rearrange("p (g c) -> p g c", g=HPC),
                )
                nc.gpsimd.tensor_copy(
                    vg[:, :, H : H + 2],
                    onescol.rearrange("p (g c) -> p g c", c=2),
                )

        def attention(tb):
            """S^T -> exp -> AV for 512-wide t-block tb, heads processed in
            pairs (partition bases 0 and 64) so the two K=64 score matmuls
            occupy disjoint PE row groups and run concurrently; one psum
            tile holds both heads' scores so a single exp covers both."""
            for hp in range(2):
                h0 = 2 * hp
                mq, mk = hp, 2 + hp
                acc_t = [
                    psp.tile([P, 4 * 66], f32, name="accp", tag=f"accp{a}",
                             bufs=1)
                    for a in range(2)
                ]
                n_s = 4 * tb + 4  # s-tiles 0 .. 4*tb+3
                for i in range(n_s):
                    first = max(0, i - 4 * tb)  # first valid jj in block
                    sps = psp.tile([P, 1024], f32, name="sp", tag="sp",
                                   bufs=2)
                    # trim fully-masked leading columns when it helps:
                    # fp32r matmuls below N=256 run at 1/4 rate, so only
                    # slice when the remaining width stays >= 256.
                    c0 = P * first if 512 - P * first >= 256 else 0
                    for hh, pb in ((0, 0), (1, 64)):
                        nc.tensor.matmul(
                            sps[:, 512 * hh + c0 : 512 * hh + 512],
                            qkt_sb[(mk, i // 4)][
                                pb : pb + H, P * (i % 4) : P * (i % 4 + 1)
                            ],
                            qkt_sb[(mq, tb)][pb : pb + H, c0:512],
                            start=True,
                            stop=True,
                        )
                    et = e_p.tile([P, 1024], avdt, name="et", tag="et")
                    if first:
                        nc.scalar.activation(
                            et.rearrange("p (g c) -> p g c", g=2)[
                                :, :, P * first : 512
                            ],
                            sps.rearrange("p (g c) -> p g c", g=2)[
                                :, :, P * first : 512
                            ],
                            mybir.ActivationFunctionType.Exp,
                            scale=1.0 / math.sqrt(H),
                        )
                    else:
                        nc.scalar.activation(
                            et,
                            sps,
                            mybir.ActivationFunctionType.Exp,
                            scale=1.0 / math.sqrt(H),
                        )
                    dj = i - 4 * tb  # diagonal jj of this s-tile, if any
                    etd = None
                    if 0 <= dj <= 3:
                        # masked diagonal sub-tiles go to a separate tile so
                        # the non-diagonal AV matmuls don't serialize behind
                        # the mask write (tile-granular dependency tracking)
                        etd = e_p.tile([P, 2 * P], avdt, name="etd", tag="etd",
                                       bufs=2)
                        for hh in range(2):
                            nc.vector.tensor_mul(
                                etd[:, P * hh : P * (hh + 1)],
                                et[:, 512 * hh + P * dj : 512 * hh + P * (dj + 1)],
                                mask,
                            )
                    for jj in range(first, 4):
                        jglob = 4 * tb + jj
                        for hh in range(2):
                            if jj == dj:
                                lhs_e = etd[:, P * hh : P * (hh + 1)]
                            else:
                                lhs_e = et[
                                    :, 512 * hh + P * jj : 512 * hh + P * (jj + 1)
                                ]
                            # start=True clears has_written for the WHOLE
                            # psum bank: only the first group per bank
                            # issues it.
                            nc.tensor.matmul(
                                acc_t[hh][:, 66 * jj : 66 * jj + 66],
                                lhs_e,
                                v_sb[i][:, G * (h0 + hh) : G * (h0 + hh) + 66],
                                start=(i == 0 and jj == 0),
                                stop=(i == jglob),
                                skip_group_check=True,
                            )
                    if 0 <= dj <= 3:
                        # acc group dj just received its last (diagonal)
                        # contribution: normalize it now so the psum bank
                        # region drains while later s-tiles still accumulate
                        for hh in range(2):
                            s0 = 66 * dj
                            rec = small_p.tile([P, 1], f32, name="rec",
                                               tag="rec")
                            nc.vector.reciprocal(
                                rec, acc_t[hh][:, s0 + H : s0 + H + 1]
                            )
                            nc.vector.tensor_scalar_mul(
                                attn_t[(tb, hp)][
                                    :, P * dj + H * hh : P * dj + H * (hh + 1)
                                ],
                                acc_t[hh][:, s0 : s0 + H],
                                rec,
                            )

        def projection(jb):
            """attn^T transposes + y = attn @ wp for 512-wide t-block jb."""
            attnT = {}
            for c in range(CD // P):
                pt = psp.tile([P, 512], mdt, name="atp", tag="wps")
                for ti in range(4):
                    nc.tensor.transpose(
                        pt[:, P * ti : P * (ti + 1)],
                        attn_t[(jb, c)][:, P * ti : P * (ti + 1)],
                        ident,
                    )
                at = e_p.tile([P, 512], mdt, name="at", tag="at", bufs=4)
                if jb == 3 and c == 0:
                    nc.scalar.copy(at, pt)
                else:
                    nc.vector.tensor_copy(at, pt)
                attnT[c] = at

            for jl in range(4):
                jt = 4 * jb + jl
                for n in range(2):
                    # block 3's projection runs after all attention: the
                    # score psum slots are free then
                    ps = psp.tile([P, 512], f32, name="yp",
                                  tag=("sp" if jb == 3 else "wps"))
                    for c in range(CD // P):
                        nc.tensor.matmul(
                            ps,
                            attnT[c][:, P * jl : P * (jl + 1)],
                            wp_sb[c][:, 512 * n : 512 * (n + 1)],
                            start=(c == 0),
                            stop=(c == CD // P - 1),
                        )
                    ysb = small_p.tile([P, 512], f32, name="ysb", tag="ysb",
                                       bufs=4)
                    if jb == 3 and (jl + n) % 2 == 0:
                        nc.scalar.copy(ysb, ps)
                    else:
                        nc.vector.tensor_copy(ysb, ps)
                    (nc.sync if (jb < 3 or n == 0) else nc.scalar).dma_start(
                        y_d[P * jt : P * (jt + 1), 512 * n : 512 * (n + 1)],
                        ysb,
                    )

        # emission order chosen so chunk-0 attention (ACT-bound) can overlap
        # the second half of phase 1 (PE-bound), and each chunk's projection
        # overlaps the next chunk's attention.
        phase1_block(0)
        attention(0)
        phase1_block(1)
        attention(1)
        phase1_block(2)
        attention(2)
        phase1_block(3)
        attention(3)
        projection(0)
        projection(1)
        projection(2)
        projection(3)

    nc.compile()
    return nc


def _get_module(mm_dt_name: str):
    if mm_dt_name not in _CACHE:
        _CACHE[mm_dt_name] = _build_module(mm_dt_name)
    return _CACHE[mm_dt_name]


def kernel(x, w_attn, b_attn, w_proj, b_proj, mm_dt_name: str = "float32r",
           trace: bool = False):
    from concourse.bass_utils import run_bass_kernel_spmd

    x = np.asarray(x, dtype=np.float32)
    w_attn = np.asarray(w_attn, dtype=np.float32)
    b_attn = np.asarray(b_attn, dtype=np.float32)
    w_proj = np.asarray(w_proj, dtype=np.float32)
    b_proj = np.asarray(b_proj, dtype=np.float32)

    nc = _get_module(mm_dt_name)

    import ml_dtypes

    avnp = np.dtype(ml_dtypes.bfloat16) if mm_dt_name == "float32r" else np.float32
    ident = np.eye(P, dtype=np.float32)
    mask = np.triu(np.ones((P, P), dtype=avnp))

    in_maps = []
    for core in range(N_CORES):
        b = core // 4
        g = core % 4
        c0 = CD * g
        wq = w_attn[:, c0 : c0 + CD]
        wk = w_attn[:, D + c0 : D + c0 + CD]
        wv = w_attn[:, 2 * D + c0 : 2 * D + c0 + CD]
        bq = b_attn[c0 : c0 + CD]
        bk = b_attn[D + c0 : D + c0 + CD]
        bvv = b_attn[2 * D + c0 : 2 * D + c0 + CD]
        in_maps.append(
            {
                "x": np.ascontiguousarray(x[b]),
                "wqk": np.ascontiguousarray(np.concatenate([wq, wk], axis=1)),
                "bqk": np.concatenate([bq, bk]).reshape(4, P).T.copy(),
                "wv": np.ascontiguousarray(wv),
                "bv": np.broadcast_to(bvv[None, :], (P, CD)).copy(),
                "wp": np.ascontiguousarray(w_proj[c0 : c0 + CD, :]),
                "ident": ident,
                "mask": mask,
                "onescol": np.tile(np.array([1.0, 0.0], avnp), (P, HPC)),
            }
        )

    res = run_bass_kernel_spmd(
        nc, in_maps, core_ids=list(range(N_CORES)), trace=trace
    )

    out = np.zeros((B, T, D), dtype=np.float32)
    for core in range(N_CORES):
        out[core // 4] += res.results[core]["y"]
    out += b_proj[None, None, :]
    if trace:
        kernel.last_result = res
    return out



# revision 7
# speedup vs baseline: 1.0232x; 1.0232x over previous
# Trainium2 Bass kernel for a causal multi-head attention block.
#
# Reference computation (fp32):
#   qkv = x @ w_attn + b_attn ; split into q,k,v heads (N=16, H=64)
#   scores = q @ k^T / sqrt(H), causal mask, softmax over keys
#   out = (weights @ v) reshaped, then out @ w_proj + b_proj
#
# Sharding: 8 cores = 2 batches x 4 head-groups (4 heads each); each core
# emits a partial [T, D] projection output; host sums the 4 head-group
# partials per batch and adds the biases.
#
# This version is built around the fp8 DoubleRow (DR) matmul mode, which
# contracts 256 rows (128 partitions x 2 interleaved k-tiles) per pass at
# 0.5 cycles/output-column -- 4x the fp32r/bf16 rate for K-bound matmuls:
#   - x^T and the qkv weights are pre-transposed / pre-quantized to fp8
#     e4m3 ON THE HOST in the DR (c p) j layout, so the kernel does zero
#     x transposes and the qkv projection runs entirely in DR mode.
#   - V is computed with an hi+lo fp8 "split" (2 extra chains) to keep its
#     quantization error negligible; Q,K run plain fp8 (their error washes
#     through the softmax).
#   - q^T/k^T are stored as fp8 e3m4 (4 mantissa bits) and the score
#     matmuls run non-DR at 1 cycle/row.
#   - exp() output is fp8 e4m3 (scaled by 8); the AV matmuls run in DR
#     mode with the SAME e tile broadcast (stride-0) across the two k-tile
#     slots against a v_hi/v_lo pair, so the V split costs nothing extra.
#     The per-head row-sum comes from a constant column in the v tiles.
#   - attn is normalized straight to bf16, transposed via the DMA XBAR
#     (no PE transposes), and the projection runs in bf16.
# All fp8 scales are powers of two folded into host prep, the psum drain
# multipliers, and the exp bias; the output DMA carries unscaled fp32.

import math

import numpy as np

B, T, D = 2, 2048, 1024
NHEAD, H = 16, 64
HPC = 4            # heads per core
CD = HPC * H       # 256 v columns per core
N_CORES = 8
P = 128
TB = 4             # t-blocks of 512
TT = 16            # t/s-tiles of 128

S_X = 16.0         # x fp8 scale
S_W = 1024.0       # w_attn fp8 scale
S_Q = 2.0          # q/k e3m4 scale
S_V = 64.0         # v fp8 scale
S_E = 8.0          # exp output scale
ALPHA = S_V        # ones-column value: attn = acc * (S_E*S_V) / (S_E*ALPHA)
QKT_MULT = S_Q / (S_X * S_W)     # 2^-13
V_MULT = S_V / (S_X * S_W)       # 2^-8
EXP_SCALE = 1.0 / (math.sqrt(H) * S_Q * S_Q)   # 2^-7
EXP_BIAS = math.log(S_E)

_CACHE = {}


def _build_module(qk_split: int = 1):
    """qk_split: number of DR chains for the q/k projection (1 = plain fp8,
    2 = + w_lo*x_hi, 3 = + w_hi*x_lo)."""
    import contextlib

    import concourse.bass as bass  # noqa: F401
    import concourse.mybir as mybir
    import concourse.tile as tile
    from concourse import bacc

    f32 = mybir.dt.float32
    bf16 = mybir.dt.bfloat16
    f8e4 = mybir.dt.float8e4
    f8e3 = mybir.dt.float8e3
    DR = mybir.MatmulPerfMode.DoubleRow
    Exp = mybir.ActivationFunctionType.Exp

    nc = bacc.Bacc("TRN2", target_bir_lowering=False, debug=False)

    # ---- DRAM I/O ----
    # x^T / weights in DR layout: row index = (c p), middle dim = j, so
    # contraction element d = 256c + 128j + p.
    xh_d = nc.dram_tensor("xh", [4 * P, 2, T], f8e4, kind="ExternalInput").ap()
    xl_d = nc.dram_tensor("xl", [4 * P, 2, T], f8e4, kind="ExternalInput").ap()
    wqkh_d = nc.dram_tensor("wqkh", [4 * P, 2, 512], f8e4, kind="ExternalInput").ap()
    wqkl_d = nc.dram_tensor("wqkl", [4 * P, 2, 512], f8e4, kind="ExternalInput").ap()
    wvh_d = nc.dram_tensor("wvh", [4 * P, 2, CD], f8e4, kind="ExternalInput").ap()
    wvl_d = nc.dram_tensor("wvl", [4 * P, 2, CD], f8e4, kind="ExternalInput").ap()
    wp_d = nc.dram_tensor("wp", [CD, D], bf16, kind="ExternalInput").ap()
    bqk_d = nc.dram_tensor("bqk", [P, 4], f32, kind="ExternalInput").ap()
    mask_d = nc.dram_tensor("mask", [P, P], f8e4, kind="ExternalInput").ap()
    vone_d = nc.dram_tensor("vone", [P, 2], f8e4, kind="ExternalInput").ap()
    y_d = nc.dram_tensor("y", [T, D], f32, kind="ExternalOutput").ap()

    with tile.TileContext(nc) as tc, contextlib.ExitStack() as ctx:
        const_p = ctx.enter_context(tc.tile_pool(name="const", bufs=1))
        w_p = ctx.enter_context(tc.tile_pool(name="weights", bufs=1))
        x_p = ctx.enter_context(tc.tile_pool(name="xin", bufs=1))
        qkt_p = ctx.enter_context(tc.tile_pool(name="qkt", bufs=1))
        v_p = ctx.enter_context(tc.tile_pool(name="vbuf", bufs=1))
        vt_p = ctx.enter_context(tc.tile_pool(name="vtmp", bufs=3))
        e_p = ctx.enter_context(tc.tile_pool(name="epool", bufs=8))
        attn_p = ctx.enter_context(tc.tile_pool(name="attn", bufs=1))
        small_p = ctx.enter_context(tc.tile_pool(name="small", bufs=8))
        y_p = ctx.enter_context(tc.tile_pool(name="ysb", bufs=4))
        # PSUM budget (8 banks of 2KB):
        #   sp   [128,1024] f32 x2   scores                     4 banks
        #   wps  [128, 512] f32 x2   qkv psum + proj psum       2 banks
        #   acc0/acc1 [128,4,65] f32 AV accumulators (2 heads)  2 banks
        psp = ctx.enter_context(tc.tile_pool(name="psp", bufs=2, space="PSUM"))

        # ---- constant / weight loads ----
        mask = const_p.tile([P, P], f8e4, name="mask_sb")
        nc.sync.dma_start(mask, mask_d)
        maskb = mask.rearrange("p (o n) -> p o n", o=1).broadcast_to([P, 2, P])
        vone = const_p.tile([P, 2], f8e4, name="vone_sb")
        nc.sync.dma_start(vone, vone_d)
        bqk = const_p.tile([P, 4], f32, name="bqk_sb")
        nc.sync.dma_start(bqk, bqk_d)
        expb = const_p.tile([P, 1], f32, name="expb")
        nc.gpsimd.memset(expb, EXP_BIAS)

        wqkh_sb, wqkl_sb, wvh_sb, wvl_sb = [], [], [], []
        for c in range(4):
            t = w_p.tile([P, 2, 512], f8e4, name=f"wqkh{c}", tag=f"wqkh{c}")
            nc.sync.dma_start(t, wqkh_d[P * c : P * (c + 1)])
            wqkh_sb.append(t)
        for c in range(4):
            t = w_p.tile([P, 2, CD], f8e4, name=f"wvh{c}", tag=f"wvh{c}")
            nc.sync.dma_start(t, wvh_d[P * c : P * (c + 1)])
            wvh_sb.append(t)
            t = w_p.tile([P, 2, CD], f8e4, name=f"wvl{c}", tag=f"wvl{c}")
            nc.scalar.dma_start(t, wvl_d[P * c : P * (c + 1)])
            wvl_sb.append(t)
        if qk_split >= 2:
            for c in range(4):
                t = w_p.tile([P, 2, 512], f8e4, name=f"wqkl{c}", tag=f"wqkl{c}")
                nc.scalar.dma_start(t, wqkl_d[P * c : P * (c + 1)])
                wqkl_sb.append(t)

        # x^T tiles: [128, 2, 2048] per c, loaded in 512-column blocks so
        # phase 1 of t-block 0 isn't gated on the whole 4MB.
        xh_sb = [x_p.tile([P, 2, T], f8e4, name=f"xh{c}", tag=f"xh{c}")
                 for c in range(4)]
        xl_sb = [x_p.tile([P, 2, T], f8e4, name=f"xl{c}", tag=f"xl{c}")
                 for c in range(4)]

        def load_x_block(j):
            sl = slice(512 * j, 512 * (j + 1))
            for c in range(4):
                nc.sync.dma_start(xh_sb[c][:, :, sl],
                                  xh_d[P * c : P * (c + 1), :, sl])
            for c in range(4):
                nc.scalar.dma_start(xl_sb[c][:, :, sl],
                                    xl_d[P * c : P * (c + 1), :, sl])

        load_x_block(0)

        wp_sb = []
        for g in range(2):
            t = w_p.tile([P, D], bf16, name=f"wp{g}", tag=f"wp{g}")
            nc.scalar.dma_start(t, wp_d[P * g : P * (g + 1), :])
            wp_sb.append(t)

        # persistent activations
        qkt_sb = {(m, j): qkt_p.tile([P, 512], f8e3, name=f"qkt{m}_{j}",
                                     tag=f"qkt{m}_{j}")
                  for m in range(4) for j in range(TB)}
        # v tiles: [s-part, 2 (hi,lo), 4 heads x 65]; col 64 of each head is
        # the row-sum column (ALPHA in hi, 0 in lo).
        v_sb = [v_p.tile([P, 2, 4 * 65], f8e4, name=f"v{i}", tag=f"v{i}")
                for i in range(TT)]
        for i in range(TT):
            vg = v_sb[i].rearrange("p j (h c) -> p j h c", h=4)
            nc.gpsimd.tensor_copy(
                vg[:, :, :, 64:65],
                vone.rearrange("p (j o c) -> p j o c", j=2, o=1)
                    .broadcast_to([P, 2, 4, 1]),
            )
        attn_sb = [attn_p.tile([P, CD], bf16, name=f"attn{jt}", tag=f"attn{jt}")
                   for jt in range(TT)]
        attnT_sb = [attn_p.tile([P, T], bf16, name=f"attnT{g}", tag=f"attnT{g}")
                    for g in range(2)]

        def bcast2(ap_2d):
            """[128, n] -> [128, 2, n] with stride-0 middle dim."""
            n = ap_2d.shape[-1]
            return ap_2d.rearrange("p (o n) -> p o n", o=1).broadcast_to([P, 2, n])

        def phase1_block(j):
            """q^T/k^T (DR fp8) + V (DR fp8 hi+lo) for 512-wide t-block j."""
            if j + 1 < TB:
                load_x_block(j + 1)
            for m in (0, 2, 1, 3):
                for u in range(2):
                    tsl = slice(512 * j + 256 * u, 512 * j + 256 * (u + 1))
                    ps = psp.tile([P, 256], f32, name="qkp", tag="wps")
                    chains = [(wqkh_sb, xh_sb)]
                    if qk_split >= 2:
                        chains.append((wqkl_sb, xh_sb))
                    if qk_split >= 3:
                        chains.append((wqkh_sb, xl_sb))
                    n_ch = len(chains)
                    for ci, (wsb, xsb) in enumerate(chains):
                        for c in range(4):
                            nc.tensor.matmul(
                                ps,
                                wsb[c][:, :, P * m : P * (m + 1)],
                                xsb[c][:, :, tsl],
                                start=(ci == 0 and c == 0),
                                stop=(ci == n_ch - 1 and c == 3),
                                perf_mode=DR,
                            )
                    nc.vector.tensor_scalar(
                        qkt_sb[(m, j)][:, 256 * u : 256 * (u + 1)], ps,
                        QKT_MULT, bqk[:, m : m + 1],
                        op0=mybir.AluOpType.mult, op1=mybir.AluOpType.add,
                    )
            for ti in range(4):
                g = 4 * j + ti
                ps = psp.tile([P, CD], f32, name="vp", tag="wps")
                xsl = slice(512 * j + P * ti, 512 * j + P * (ti + 1))
                for ci, wsb in enumerate((wvh_sb, wvl_sb)):
                    for c in range(4):
                        nc.tensor.matmul(
                            ps,
                            xh_sb[c][:, :, xsl],
                            wsb[c],
                            start=(ci == 0 and c == 0),
                            stop=False,
                            perf_mode=DR,
                        )
                for c in range(4):
                    nc.tensor.matmul(
                        ps,
                        xl_sb[c][:, :, xsl],
                        wvh_sb[c],
                        start=False,
                        stop=(c == 3),
                        perf_mode=DR,
                    )
                vtmp = vt_p.tile([P, CD], f32, name="vtmp", tag="vtmp")
                nc.vector.tensor_scalar_mul(vtmp, ps, V_MULT)
                vg = v_sb[g].rearrange("p j (h c) -> p j h c", h=4)
                vtg = vtmp.rearrange("p (h c) -> p h c", h=4)
                nc.gpsimd.tensor_copy(vg[:, 0, :, 0:64], vtg)
                nc.gpsimd.tensor_tensor(
                    vg[:, 1, :, 0:64], vtg, vg[:, 0, :, 0:64],
                    op=mybir.AluOpType.subtract,
                )

        def attention(tb):
            """scores (e3m4) -> exp (e4m3) -> AV (DR, hi+lo via stride-0
            broadcast of e) for 512-wide t-block tb, one head-pair at a
            time so only 2 psum accumulator banks are live."""
            for hp in range(2):
                mq, mk = hp, 2 + hp
                acc = [psp.tile([P, 4, 65], f32, name=f"acc{hh}",
                                tag=f"acc{hh}", bufs=1) for hh in range(2)]
                n_s = 4 * tb + 4
                for i in range(n_s):
                    first = max(0, i - 4 * tb)
                    c0 = P * first
                    sps = psp.tile([P, 1024], f32, name="sp", tag="sp", bufs=2)
                    for hh, pb in ((0, 0), (1, 64)):
                        nc.tensor.matmul(
                            sps[:, 512 * hh + c0 : 512 * hh + 512],
                            qkt_sb[(mk, i // 4)][pb : pb + H,
                                                 P * (i % 4) : P * (i % 4 + 1)],
                            qkt_sb[(mq, tb)][pb : pb + H, c0:512],
                            start=True,
                            stop=True,
                        )
                    et = e_p.tile([P, 1024], f8e4, name="et", tag="et")
                    if first:
                        nc.scalar.activation(
                            et.rearrange("p (g c) -> p g c", g=2)[:, :, c0:512],
                            sps.rearrange("p (g c) -> p g c", g=2)[:, :, c0:512],
                            Exp, scale=EXP_SCALE, bias=expb,
                        )
                    else:
                        nc.scalar.activation(et, sps, Exp, scale=EXP_SCALE,
                                             bias=expb)
                    dj = i - 4 * tb
                    etd = None
                    if 0 <= dj <= 3:
                        etd = e_p.tile([P, 2 * P], f8e4, name="etd", tag="etd",
                                       bufs=2)
                        nc.gpsimd.tensor_tensor(
                            etd.rearrange("p (g c) -> p g c", g=2),
                            et.rearrange("p (g c) -> p g c", g=2)[
                                :, :, P * dj : P * (dj + 1)],
                            maskb,
                            op=mybir.AluOpType.mult,
                        )
                    for jj in range(first, 4):
                        for hh in range(2):
                            if jj == dj:
                                lhs_e = bcast2(etd[:, P * hh : P * (hh + 1)])
                            else:
                                lhs_e = bcast2(
                                    et[:, 512 * hh + P * jj : 512 * hh + P * (jj + 1)]
                                )
                            h = 2 * hp + hh
                            nc.tensor.matmul(
                                acc[hh][:, jj, :],
                                lhs_e,
                                v_sb[i][:, :, 65 * h : 65 * (h + 1)],
                                start=(i == 0 and jj == 0),
                                stop=(i == 4 * tb + jj),
                                perf_mode=DR,
                                skip_group_check=True,
                            )
                    if 0 <= dj <= 3:
                        jt = 4 * tb + dj
                        for hh in range(2):
                            h = 2 * hp + hh
                            rec = small_p.tile([P, 1], f32, name="rec", tag="rec")
                            nc.vector.reciprocal(rec, acc[hh][:, dj, 64:65])
                            nc.vector.tensor_scalar_mul(
                                attn_sb[jt][:, H * h : H * (h + 1)],
                                acc[hh][:, dj, 0:64],
                                rec,
                            )
                # attn columns for this head pair are final: DMA-transpose
                # them into attnT (XBAR), per 128x128 block.
                for dj in range(4):
                    jt = 4 * tb + dj
                    nc.sync.dma_start(
                        attnT_sb[hp][:, P * jt : P * (jt + 1)],
                        attn_sb[jt][:, P * hp : P * (hp + 1)],
                        transpose=True,
                    )

        def projection(tb):
            """y[512 t-block] = attn @ wp in bf16 (K=256 over 2 groups)."""
            for jl in range(4):
                jt = 4 * tb + jl
                for n in range(2):
                    ps = psp.tile([P, 512], f32, name="yp", tag="wps")
                    for g in range(2):
                        nc.tensor.matmul(
                            ps,
                            attnT_sb[g][:, P * jt : P * (jt + 1)],
                            wp_sb[g][:, 512 * n : 512 * (n + 1)],
                            start=(g == 0),
                            stop=(g == 1),
                        )
                    ysb = y_p.tile([P, 512], f32, name="ysb", tag="ysb")
                    nc.vector.tensor_copy(ysb, ps)
                    nc.sync.dma_start(
                        y_d[P * jt : P * (jt + 1), 512 * n : 512 * (n + 1)],
                        ysb,
                    )

        phase1_block(0)
        attention(0)
        phase1_block(1)
        attention(1)
        projection(0)
        phase1_block(2)
        attention(2)
        projection(1)
        phase1_block(3)
        attention(3)
        projection(2)
        projection(3)

    nc.compile()
    return nc


def _get_module(qk_split: int = 1):
    if qk_split not in _CACHE:
        _CACHE[qk_split] = _build_module(qk_split)
    return _CACHE[qk_split]


def _dr_layout(a):
    """[1024, n] -> [(c p), j, n] with row d = 256c + 128j + p."""
    n = a.shape[1]
    return np.ascontiguousarray(
        a.reshape(4, 2, P, n).transpose(0, 2, 1, 3).reshape(4 * P, 2, n)
    )


def kernel(x, w_attn, b_attn, w_proj, b_proj, qk_split: int = 1,
           trace: bool = False):
    import ml_dtypes
    from concourse.bass_utils import run_bass_kernel_spmd

    e4 = np.dtype(ml_dtypes.float8_e4m3)  # IEEE variant: max 240, like the HW
    e3 = np.dtype(ml_dtypes.float8_e3m4)
    bf = np.dtype(ml_dtypes.bfloat16)

    x = np.asarray(x, dtype=np.float32)
    w_attn = np.asarray(w_attn, dtype=np.float32)
    b_attn = np.asarray(b_attn, dtype=np.float32)
    w_proj = np.asarray(w_proj, dtype=np.float32)
    b_proj = np.asarray(b_proj, dtype=np.float32)

    nc = _get_module(qk_split)

    mask = np.triu(np.ones((P, P), dtype=np.float32)).astype(e4)
    vone = np.zeros((P, 2), dtype=np.float32)
    vone[:, 0] = ALPHA
    vone = vone.astype(e4)

    # per-batch x prep
    x_prep = []
    for b in range(B):
        x16 = x[b].T * S_X                      # [1024, 2048]
        xh = x16.astype(e4)
        xl = (x16 - xh.astype(np.float32)).astype(e4)
        x_prep.append((_dr_layout(xh), _dr_layout(xl)))

    in_maps = []
    for core in range(N_CORES):
        b, g = core // 4, core % 4
        c0 = CD * g
        wq = w_attn[:, c0 : c0 + CD]
        wk = w_attn[:, D + c0 : D + c0 + CD]
        wv = w_attn[:, 2 * D + c0 : 2 * D + c0 + CD]
        wqk = np.concatenate([wq, wk], axis=1) * S_W
        wqkh = wqk.astype(e4)
        wqkl = (wqk - wqkh.astype(np.float32)).astype(e4)
        wv_s = wv * S_W
        wvh = wv_s.astype(e4)
        wvl = (wv_s - wvh.astype(np.float32)).astype(e4)
        bq = b_attn[c0 : c0 + CD]
        bk = b_attn[D + c0 : D + c0 + CD]
        xh, xl = x_prep[b]
        in_maps.append(
            {
                "xh": xh,
                "xl": xl,
                "wqkh": _dr_layout(wqkh),
                "wqkl": _dr_layout(wqkl),
                "wvh": _dr_layout(wvh),
                "wvl": _dr_layout(wvl),
                "wp": np.ascontiguousarray(w_proj[c0 : c0 + CD, :]).astype(bf),
                "bqk": (np.concatenate([bq, bk]) * S_Q).reshape(4, P).T.copy(),
                "mask": mask,
                "vone": vone,
            }
        )

    res = run_bass_kernel_spmd(
        nc, in_maps, core_ids=list(range(N_CORES)), trace=trace
    )

    out = np.zeros((B, T, D), dtype=np.float32)
    for core in range(N_CORES):
        out[core // 4] += np.asarray(res.results[core]["y"], dtype=np.float32)
    out += (b_proj + b_attn[2 * D :] @ w_proj)[None, None, :]
    if trace:
        kernel.last_result = res
    return out


# revision 9
# speedup vs baseline: 1.0711x; 1.0468x over previous
# Trainium2 Bass kernel for a causal multi-head attention block.
#
# Reference computation (fp32):
#   qkv = x @ w_attn + b_attn ; split into q,k,v heads (N=16, H=64)
#   scores = q @ k^T / sqrt(H), causal mask, softmax over keys
#   out = (weights @ v) reshaped, then out @ w_proj + b_proj
#
# Sharding: 8 cores = 2 batches x 4 head-groups (4 heads each); each core
# emits a partial [T, D] projection output; host sums the 4 head-group
# partials per batch and adds the biases.
#
# This version is built around the fp8 DoubleRow (DR) matmul mode, which
# contracts 256 rows (128 partitions x 2 interleaved k-tiles) per pass at
# 0.5 cycles/output-column -- 4x the fp32r/bf16 rate for K-bound matmuls:
#   - x^T and the qkv weights are pre-transposed / pre-quantized to fp8
#     e4m3 ON THE HOST in the DR (c p) j layout, so the kernel does zero
#     x transposes and the qkv projection runs entirely in DR mode.
#   - V is computed with an hi+lo fp8 "split" (2 extra chains) to keep its
#     quantization error negligible; Q,K run plain fp8 (their error washes
#     through the softmax).
#   - q^T/k^T are stored as fp8 e3m4 (4 mantissa bits) and the score
#     matmuls run non-DR at 1 cycle/row.
#   - exp() output is fp8 e4m3 (scaled by 8); the AV matmuls run in DR
#     mode with the SAME e tile broadcast (stride-0) across the two k-tile
#     slots against a v_hi/v_lo pair, so the V split costs nothing extra.
#     The per-head row-sum comes from a constant column in the v tiles.
#   - attn is normalized straight to bf16, transposed via the DMA XBAR
#     (no PE transposes), and the projection runs in bf16.
# All fp8 scales are powers of two folded into host prep, the psum drain
# multipliers, and the exp bias; the output DMA carries unscaled fp32.

import math

import numpy as np

B, T, D = 2, 2048, 1024
NHEAD, H = 16, 64
HPC = 4            # heads per core
CD = HPC * H       # 256 v columns per core
N_CORES = 8
P = 128
TB = 4             # t-blocks of 512
TT = 16            # t/s-tiles of 128

S_X = 16.0         # x fp8 scale
S_W = 1024.0       # w_attn fp8 scale
S_Q = 2.0          # q/k e3m4 scale
S_V = 64.0         # v fp8 scale
S_E = 8.0          # exp output scale
ALPHA = S_V        # ones-column value: attn = acc * (S_E*S_V) / (S_E*ALPHA)
QKT_MULT = S_Q / (S_X * S_W)     # 2^-13
V_MULT = S_V / (S_X * S_W)       # 2^-8
EXP_SCALE = 1.0 / (math.sqrt(H) * S_Q * S_Q)   # 2^-7
EXP_BIAS = math.log(S_E)

_CACHE = {}


def _build_module(qk_split: int = 1):
    """qk_split: number of DR chains for the q/k projection (1 = plain fp8,
    2 = + w_lo*x_hi, 3 = + w_hi*x_lo)."""
    import contextlib

    import concourse.bass as bass  # noqa: F401
    import concourse.mybir as mybir
    import concourse.tile as tile
    from concourse import bacc

    f32 = mybir.dt.float32
    bf16 = mybir.dt.bfloat16
    f8e4 = mybir.dt.float8e4
    f8e3 = mybir.dt.float8e3
    DR = mybir.MatmulPerfMode.DoubleRow
    Exp = mybir.ActivationFunctionType.Exp

    nc = bacc.Bacc("TRN2", target_bir_lowering=False, debug=False)

    # ---- DRAM I/O ----
    # x^T / weights in DR layout: row index = (c p), middle dim = j, so
    # contraction element d = 256c + 128j + p.
    xh_d = nc.dram_tensor("xh", [4 * P, 2, T], f8e4, kind="ExternalInput").ap()
    xl_d = nc.dram_tensor("xl", [4 * P, 2, T], f8e4, kind="ExternalInput").ap()
    wqkh_d = nc.dram_tensor("wqkh", [4 * P, 2, 512], f8e4, kind="ExternalInput").ap()
    wqkl_d = nc.dram_tensor("wqkl", [4 * P, 2, 512], f8e4, kind="ExternalInput").ap()
    wvh_d = nc.dram_tensor("wvh", [4 * P, 2, CD], f8e4, kind="ExternalInput").ap()
    wvl_d = nc.dram_tensor("wvl", [4 * P, 2, CD], f8e4, kind="ExternalInput").ap()
    wp_d = nc.dram_tensor("wp", [CD, D], bf16, kind="ExternalInput").ap()
    bqk_d = nc.dram_tensor("bqk", [P, 4], f32, kind="ExternalInput").ap()
    mask_d = nc.dram_tensor("mask", [P, P], f8e4, kind="ExternalInput").ap()
    vone_d = nc.dram_tensor("vone", [P, 2], f8e4, kind="ExternalInput").ap()
    y_d = nc.dram_tensor("y", [T, D], f32, kind="ExternalOutput").ap()

    with tile.TileContext(nc) as tc, contextlib.ExitStack() as ctx:
        const_p = ctx.enter_context(tc.tile_pool(name="const", bufs=1))
        w_p = ctx.enter_context(tc.tile_pool(name="weights", bufs=1))
        x_p = ctx.enter_context(tc.tile_pool(name="xin", bufs=1))
        qkt_p = ctx.enter_context(tc.tile_pool(name="qkt", bufs=1))
        v_p = ctx.enter_context(tc.tile_pool(name="vbuf", bufs=1))
        vt_p = ctx.enter_context(tc.tile_pool(name="vtmp", bufs=3))
        e_p = ctx.enter_context(tc.tile_pool(name="epool", bufs=8))
        attn_p = ctx.enter_context(tc.tile_pool(name="attn", bufs=1))
        small_p = ctx.enter_context(tc.tile_pool(name="small", bufs=8))
        y_p = ctx.enter_context(tc.tile_pool(name="ysb", bufs=4))
        # PSUM budget (8 banks of 2KB):
        #   sp   [128,1024] f32 x2   scores                     4 banks
        #   wps  [128, 512] f32 x2   qkv psum + proj psum       2 banks
        #   acc0/acc1 [128,4,65] f32 AV accumulators (2 heads)  2 banks
        psp = ctx.enter_context(tc.tile_pool(name="psp", bufs=2, space="PSUM"))

        # ---- constant / weight loads ----
        mask = const_p.tile([P, P], f8e4, name="mask_sb")
        nc.sync.dma_start(mask, mask_d)
        maskb = mask.rearrange("p (o n) -> p o n", o=1).broadcast_to([P, 2, P])
        vone = const_p.tile([P, 2], f8e4, name="vone_sb")
        nc.sync.dma_start(vone, vone_d)
        bqk = const_p.tile([P, 4], f32, name="bqk_sb")
        nc.sync.dma_start(bqk, bqk_d)
        expb = const_p.tile([P, 1], f32, name="expb")
        nc.gpsimd.memset(expb, EXP_BIAS)

        wqkh_sb, wqkl_sb, wvh_sb, wvl_sb = [], [], [], []
        for c in range(4):
            t = w_p.tile([P, 2, 512], f8e4, name=f"wqkh{c}", tag=f"wqkh{c}")
            nc.sync.dma_start(t, wqkh_d[P * c : P * (c + 1)])
            wqkh_sb.append(t)
        for c in range(4):
            t = w_p.tile([P, 2, CD], f8e4, name=f"wvh{c}", tag=f"wvh{c}")
            nc.sync.dma_start(t, wvh_d[P * c : P * (c + 1)])
            wvh_sb.append(t)
            t = w_p.tile([P, 2, CD], f8e4, name=f"wvl{c}", tag=f"wvl{c}")
            nc.scalar.dma_start(t, wvl_d[P * c : P * (c + 1)])
            wvl_sb.append(t)
        if qk_split >= 2:
            for c in range(4):
                t = w_p.tile([P, 2, 512], f8e4, name=f"wqkl{c}", tag=f"wqkl{c}")
                nc.scalar.dma_start(t, wqkl_d[P * c : P * (c + 1)])
                wqkl_sb.append(t)

        # x^T tiles: [128, 2, 2048] per c, loaded in 512-column blocks so
        # phase 1 of t-block 0 isn't gated on the whole 4MB.
        xh_sb = [x_p.tile([P, 2, T], f8e4, name=f"xh{c}", tag=f"xh{c}")
                 for c in range(4)]
        xl_sb = [x_p.tile([P, 2, T], f8e4, name=f"xl{c}", tag=f"xl{c}")
                 for c in range(4)]

        def load_x_block(j):
            sl = slice(512 * j, 512 * (j + 1))
            for c in range(4):
                nc.sync.dma_start(xh_sb[c][:, :, sl],
                                  xh_d[P * c : P * (c + 1), :, sl])
            for c in range(4):
                nc.scalar.dma_start(xl_sb[c][:, :, sl],
                                    xl_d[P * c : P * (c + 1), :, sl])

        load_x_block(0)

        wp_sb = []
        for g in range(2):
            t = w_p.tile([P, D], bf16, name=f"wp{g}", tag=f"wp{g}")
            nc.scalar.dma_start(t, wp_d[P * g : P * (g + 1), :])
            wp_sb.append(t)

        # persistent activations
        qkt_sb = {(m, j): qkt_p.tile([P, 512], f8e3, name=f"qkt{m}_{j}",
                                     tag=f"qkt{m}_{j}")
                  for m in range(4) for j in range(TB)}
        # v tiles: [s-part, 2 (hi,lo), 4 heads x 65]; col 64 of each head is
        # the row-sum column (ALPHA in hi, 0 in lo).
        v_sb = [v_p.tile([P, 2, 4 * 65], f8e4, name=f"v{i}", tag=f"v{i}")
                for i in range(TT)]
        for i in range(TT):
            vg = v_sb[i].rearrange("p j (h c) -> p j h c", h=4)
            nc.gpsimd.tensor_copy(
                vg[:, :, :, 64:65],
                vone.rearrange("p (j o c) -> p j o c", j=2, o=1)
                    .broadcast_to([P, 2, 4, 1]),
            )
        attn_sb = [attn_p.tile([P, CD], bf16, name=f"attn{jt}", tag=f"attn{jt}")
                   for jt in range(TT)]
        attnT_sb = [attn_p.tile([P, T], bf16, name=f"attnT{g}", tag=f"attnT{g}")
                    for g in range(2)]

        def bcast2(ap_2d):
            """[128, n] -> [128, 2, n] with stride-0 middle dim."""
            n = ap_2d.shape[-1]
            return ap_2d.rearrange("p (o n) -> p o n", o=1).broadcast_to([P, 2, n])

        def qk_chunk(j, m, u):
            """q^T/k^T (DR fp8) for m-slice m, 256-col half u of t-block j."""
            tsl = slice(512 * j + 256 * u, 512 * j + 256 * (u + 1))
            ps = psp.tile([P, 256], f32, name="qkp", tag="wps")
            chains = [(wqkh_sb, xh_sb)]
            if qk_split >= 2:
                chains.append((wqkl_sb, xh_sb))
            if qk_split >= 3:
                chains.append((wqkh_sb, xl_sb))
            n_ch = len(chains)
            for ci, (wsb, xsb) in enumerate(chains):
                for c in range(4):
                    nc.tensor.matmul(
                        ps,
                        wsb[c][:, :, P * m : P * (m + 1)],
                        xsb[c][:, :, tsl],
                        start=(ci == 0 and c == 0),
                        stop=(ci == n_ch - 1 and c == 3),
                        perf_mode=DR,
                    )
            nc.vector.tensor_scalar(
                qkt_sb[(m, j)][:, 256 * u : 256 * (u + 1)], ps,
                QKT_MULT, bqk[:, m : m + 1],
                op0=mybir.AluOpType.mult, op1=mybir.AluOpType.add,
            )

        def v_chunk(j, ti):
            """V (DR fp8 hi+lo split) for s-tile 4j+ti."""
            g = 4 * j + ti
            ps = psp.tile([P, CD], f32, name="vp", tag="wps")
            xsl = slice(512 * j + P * ti, 512 * j + P * (ti + 1))
            for ci, wsb in enumerate((wvh_sb, wvl_sb)):
                for c in range(4):
                    nc.tensor.matmul(
                        ps, xh_sb[c][:, :, xsl], wsb[c],
                        start=(ci == 0 and c == 0), stop=False, perf_mode=DR,
                    )
            for c in range(4):
                nc.tensor.matmul(
                    ps, xl_sb[c][:, :, xsl], wvh_sb[c],
                    start=False, stop=(c == 3), perf_mode=DR,
                )
            vtmp = vt_p.tile([P, CD], f32, name="vtmp", tag="vtmp")
            nc.vector.tensor_scalar_mul(vtmp, ps, V_MULT)
            vg = v_sb[g].rearrange("p j (h c) -> p j h c", h=4)
            vtg = vtmp.rearrange("p (h c) -> p h c", h=4)
            nc.gpsimd.tensor_copy(vg[:, 0, :, 0:64], vtg)
            nc.gpsimd.tensor_tensor(
                vg[:, 1, :, 0:64], vtg, vg[:, 0, :, 0:64],
                op=mybir.AluOpType.subtract,
            )

        def phase1_chunks(j):
            return ([(lambda m=m, u=u: qk_chunk(j, m, u))
                     for m in (0, 2, 1, 3) for u in range(2)]
                    + [(lambda ti=ti: v_chunk(j, ti)) for ti in range(4)])

        def proj_chunk(jt, n):
            """y[t-tile jt, 512-col half n] = attn @ wp in bf16."""
            ps = psp.tile([P, 512], f32, name="yp", tag="wps")
            for g in range(2):
                nc.tensor.matmul(
                    ps,
                    attnT_sb[g][:, P * jt : P * (jt + 1)],
                    wp_sb[g][:, 512 * n : 512 * (n + 1)],
                    start=(g == 0),
                    stop=(g == 1),
                )
            ysb = y_p.tile([P, 512], f32, name="ysb", tag="ysb")
            nc.vector.tensor_copy(ysb, ps)
            nc.sync.dma_start(
                y_d[P * jt : P * (jt + 1), 512 * n : 512 * (n + 1)], ysb,
            )

        def proj_chunks(tb):
            return [(lambda jt=4 * tb + jl, n=n: proj_chunk(jt, n))
                    for jl in range(4) for n in range(2)]

        def attention(tb, fillers):
            """scores (e3m4) -> exp (e4m3) -> AV (DR, hi+lo via stride-0
            broadcast of e) for 512-wide t-block tb, one head-pair at a
            time so only 2 psum accumulator banks are live.

            Software-pipelined: the AV matmuls for s-tile i are emitted two
            iterations late so the in-order PE queue never stalls waiting
            for exp(i); `fillers` (qkv of the next block / projection of
            the previous one) are interleaved to keep the PE busy and at
            full p-state while the ACT engine grinds through the exps."""
            n_s = 4 * tb + 4
            LAG = 2
            for hp in range(2):
                mq, mk = hp, 2 + hp
                acc = [psp.tile([P, 4, 65], f32, name=f"acc{hh}",
                                tag=f"acc{hh}", bufs=1) for hh in range(2)]
                ets = {}

                def front(i):
                    first = max(0, i - 4 * tb)
                    c0 = P * first
                    sps = psp.tile([P, 1024], f32, name="sp", tag="sp", bufs=2)
                    for hh, pb in ((0, 0), (1, 64)):
                        nc.tensor.matmul(
                            sps[:, 512 * hh + c0 : 512 * hh + 512],
                            qkt_sb[(mk, i // 4)][pb : pb + H,
                                                 P * (i % 4) : P * (i % 4 + 1)],
                            qkt_sb[(mq, tb)][pb : pb + H, c0:512],
                            start=True,
                            stop=True,
                        )
                    et = e_p.tile([P, 1024], f8e4, name="et", tag="et")
                    if first:
                        nc.scalar.activation(
                            et.rearrange("p (g c) -> p g c", g=2)[:, :, c0:512],
                            sps.rearrange("p (g c) -> p g c", g=2)[:, :, c0:512],
                            Exp, scale=EXP_SCALE, bias=expb,
                        )
                    else:
                        nc.scalar.activation(et, sps, Exp, scale=EXP_SCALE,
                                             bias=expb)
                    dj = i - 4 * tb
                    etd = None
                    if 0 <= dj <= 3:
                        etd = e_p.tile([P, 2 * P], f8e4, name="etd", tag="etd",
                                       bufs=2)
                        nc.gpsimd.tensor_tensor(
                            etd.rearrange("p (g c) -> p g c", g=2),
                            et.rearrange("p (g c) -> p g c", g=2)[
                                :, :, P * dj : P * (dj + 1)],
                            maskb,
                            op=mybir.AluOpType.mult,
                        )
                    ets[i] = (et, etd)

                def back(i):
                    first = max(0, i - 4 * tb)
                    dj = i - 4 * tb
                    et, etd = ets.pop(i)
                    for jj in range(first, 4):
                        for hh in range(2):
                            if jj == dj:
                                lhs_e = bcast2(etd[:, P * hh : P * (hh + 1)])
                            else:
                                lhs_e = bcast2(
                                    et[:, 512 * hh + P * jj : 512 * hh + P * (jj + 1)]
                                )
                            h = 2 * hp + hh
                            nc.tensor.matmul(
                                acc[hh][:, jj, :],
                                lhs_e,
                                v_sb[i][:, :, 65 * h : 65 * (h + 1)],
                                start=(i == 0 and jj == 0),
                                stop=(i == 4 * tb + jj),
                                perf_mode=DR,
                                skip_group_check=True,
                            )
                    if 0 <= dj <= 3:
                        jt = 4 * tb + dj
                        for hh in range(2):
                            h = 2 * hp + hh
                            rec = small_p.tile([P, 1], f32, name="rec", tag="rec")
                            nc.vector.reciprocal(rec, acc[hh][:, dj, 64:65])
                            nc.vector.tensor_scalar_mul(
                                attn_sb[jt][:, H * h : H * (h + 1)],
                                acc[hh][:, dj, 0:64],
                                rec,
                            )

                for i in range(n_s):
                    front(i)
                    if fillers:
                        fillers.pop(0)()
                    if i >= LAG:
                        back(i - LAG)
                for i in range(n_s - LAG, n_s):
                    back(i)
                # attn columns for this head pair are final: DMA-transpose
                # them into attnT (XBAR), per 128x128 block.
                for dj in range(4):
                    jt = 4 * tb + dj
                    nc.sync.dma_start(
                        attnT_sb[hp][:, P * jt : P * (jt + 1)],
                        attn_sb[jt][:, P * hp : P * (hp + 1)],
                        transpose=True,
                    )
            while fillers:
                fillers.pop(0)()

        for ch in phase1_chunks(0):
            ch()
        load_x_block(1)
        attention(0, phase1_chunks(1))
        load_x_block(2)
        attention(1, proj_chunks(0) + phase1_chunks(2))
        load_x_block(3)
        attention(2, proj_chunks(1) + phase1_chunks(3))
        attention(3, proj_chunks(2))
        for ch in proj_chunks(3):
            ch()

    nc.compile()
    return nc


def _get_module(qk_split: int = 1):
    if qk_split not in _CACHE:
        _CACHE[qk_split] = _build_module(qk_split)
    return _CACHE[qk_split]


def _dr_layout(a):
    """[1024, n] -> [(c p), j, n] with row d = 256c + 128j + p."""
    n = a.shape[1]
    return np.ascontiguousarray(
        a.reshape(4, 2, P, n).transpose(0, 2, 1, 3).reshape(4 * P, 2, n)
    )


def kernel(x, w_attn, b_attn, w_proj, b_proj, qk_split: int = 1,
           trace: bool = False):
    import ml_dtypes
    from concourse.bass_utils import run_bass_kernel_spmd

    e4 = np.dtype(ml_dtypes.float8_e4m3)  # IEEE variant: max 240, like the HW
    e3 = np.dtype(ml_dtypes.float8_e3m4)
    bf = np.dtype(ml_dtypes.bfloat16)

    x = np.asarray(x, dtype=np.float32)
    w_attn = np.asarray(w_attn, dtype=np.float32)
    b_attn = np.asarray(b_attn, dtype=np.float32)
    w_proj = np.asarray(w_proj, dtype=np.float32)
    b_proj = np.asarray(b_proj, dtype=np.float32)

    nc = _get_module(qk_split)

    mask = np.triu(np.ones((P, P), dtype=np.float32)).astype(e4)
    vone = np.zeros((P, 2), dtype=np.float32)
    vone[:, 0] = ALPHA
    vone = vone.astype(e4)

    # per-batch x prep
    x_prep = []
    for b in range(B):
        x16 = x[b].T * S_X                      # [1024, 2048]
        xh = x16.astype(e4)
        xl = (x16 - xh.astype(np.float32)).astype(e4)
        x_prep.append((_dr_layout(xh), _dr_layout(xl)))

    in_maps = []
    for core in range(N_CORES):
        b, g = core // 4, core % 4
        c0 = CD * g
        wq = w_attn[:, c0 : c0 + CD]
        wk = w_attn[:, D + c0 : D + c0 + CD]
        wv = w_attn[:, 2 * D + c0 : 2 * D + c0 + CD]
        wqk = np.concatenate([wq, wk], axis=1) * S_W
        wqkh = wqk.astype(e4)
        wqkl = (wqk - wqkh.astype(np.float32)).astype(e4)
        wv_s = wv * S_W
        wvh = wv_s.astype(e4)
        wvl = (wv_s - wvh.astype(np.float32)).astype(e4)
        bq = b_attn[c0 : c0 + CD]
        bk = b_attn[D + c0 : D + c0 + CD]
        xh, xl = x_prep[b]
        in_maps.append(
            {
                "xh": xh,
                "xl": xl,
                "wqkh": _dr_layout(wqkh),
                "wqkl": _dr_layout(wqkl),
                "wvh": _dr_layout(wvh),
                "wvl": _dr_layout(wvl),
                "wp": np.ascontiguousarray(w_proj[c0 : c0 + CD, :]).astype(bf),
                "bqk": (np.concatenate([bq, bk]) * S_Q).reshape(4, P).T.copy(),
                "mask": mask,
                "vone": vone,
            }
        )

    res = run_bass_kernel_spmd(
        nc, in_maps, core_ids=list(range(N_CORES)), trace=trace
    )

    out = np.zeros((B, T, D), dtype=np.float32)
    for core in range(N_CORES):
        out[core // 4] += np.asarray(res.results[core]["y"], dtype=np.float32)
    out += (b_proj + b_attn[2 * D :] @ w_proj)[None, None, :]
    if trace:
        kernel.last_result = res
    return out


# revision 10
# speedup vs baseline: 1.1812x; 1.1029x over previous
# Trainium2 Bass kernel for a causal multi-head attention block.
#
# Reference computation (fp32):
#   qkv = x @ w_attn + b_attn ; split into q,k,v heads (N=16, H=64)
#   scores = q @ k^T / sqrt(H), causal mask, softmax over keys
#   out = (weights @ v) reshaped, then out @ w_proj + b_proj
#
# Sharding: 8 cores = 2 batches x 4 head-groups (4 heads each); each core
# emits a partial [T, D] projection output; host sums the 4 head-group
# partials per batch and adds the biases.
#
# The kernel is built around the fp8 DoubleRow (DR) matmul mode, which
# contracts 256 rows (128 partitions x 2 interleaved k-tiles) per pass at
# 0.5 cycles/output-column -- 4x the fp32r/bf16 rate for K-bound matmuls:
#   - x^T and the qkv weights are pre-transposed / pre-quantized to fp8
#     e4m3 (IEEE: max 240) ON THE HOST in the DR (p)(c j) layout, so the
#     kernel does zero x transposes and qkv runs entirely in DR mode.
#   - V is computed with an hi+lo fp8 split (2 extra chains) to keep its
#     quantization error negligible; Q,K run plain fp8 (their error washes
#     through the softmax).
#   - q^T/k^T are stored as fp8 e3m4 (4 mantissa bits, max 15.5) and the
#     score matmuls run non-DR at 1 cycle/row.
#   - exp() output is fp8 e4m3 (scaled by 8); the AV matmuls run in DR
#     mode with the SAME e tile broadcast (stride-0) across the two k-tile
#     slots against a v_hi/v_lo pair, so the V split costs nothing extra.
#     The per-head row-sum comes from a constant column in the v tiles.
#   - attn is normalized straight to bf16, transposed via the DMA XBAR
#     (no PE transposes), and the projection runs in bf16.
# All fp8 scales are powers of two folded into host prep, the psum drain
# multipliers, and the exp bias; the output DMA carries unscaled fp32.
#
# Scheduling: the ACT engine (exp over the causal half of the score
# matrix) is the roofline at ~60us busy; everything else is arranged
# around keeping its queue full: the AV matmuls trail the score/exp
# stream by 2 s-tiles (the in-order PE queue must never park on an
# exp-dependent instruction), the qkv matmuls of the next t-block and
# the projection of the previous one are chopped into filler chunks
# emitted between score iterations, and the two head-pair streams of
# each t-block are fused into one front/back pipeline.  DMAs are merged
# (one per x block / weight tensor, one per output t-tile) because each
# HWDGE issue costs ~625ns of shared descriptor-generation time; the
# output DMAs ride the gpsimd SWDGE path to keep HWDGE free.

import math

import numpy as np

B, T, D = 2, 2048, 1024
NHEAD, H = 16, 64
HPC = 4            # heads per core
CD = HPC * H       # 256 v columns per core
N_CORES = 8
P = 128
TB = 4             # t-blocks of 512
TT = 16            # t/s-tiles of 128

S_X = 16.0         # x fp8 scale
S_W = 1024.0       # w_attn fp8 scale
S_Q = 2.0          # q/k e3m4 scale
S_V = 64.0         # v fp8 scale
S_E = 8.0          # exp output scale
ALPHA = S_V        # ones-column value: attn = acc * (S_E*S_V) / (S_E*ALPHA)
QKT_MULT = S_Q / (S_X * S_W)
V_MULT = S_V / (S_X * S_W)
EXP_SCALE = 1.0 / (math.sqrt(H) * S_Q * S_Q)
EXP_BIAS = math.log(S_E)

_CACHE = {}


def _build_module(qk_split: int = 1):
    """qk_split: number of DR chains for the q/k projection (1 = plain fp8,
    2 = + w_lo*x_hi, 3 = + w_hi*x_lo)."""
    import contextlib

    import concourse.bass as bass  # noqa: F401
    import concourse.mybir as mybir
    import concourse.tile as tile
    from concourse import bacc

    f32 = mybir.dt.float32
    bf16 = mybir.dt.bfloat16
    f8e4 = mybir.dt.float8e4
    f8e3 = mybir.dt.float8e3
    DR = mybir.MatmulPerfMode.DoubleRow
    Exp = mybir.ActivationFunctionType.Exp

    nc = bacc.Bacc("TRN2", target_bir_lowering=False, debug=False)

    # ---- DRAM I/O ----
    # x^T: [p][blk][c][j][t512], so contraction element d = 256c + 128j + p
    # and each 512-t block is one 4KB-per-partition contiguous DMA.
    xh_d = nc.dram_tensor("xh", [P, TB, 4, 2, 512], f8e4, kind="ExternalInput").ap()
    xl_d = nc.dram_tensor("xl", [P, TB, 4, 2, 512], f8e4, kind="ExternalInput").ap()
    # weights: [p][c][j][cols]
    wqkh_d = nc.dram_tensor("wqkh", [P, 4, 2, 512], f8e4, kind="ExternalInput").ap()
    wqkl_d = nc.dram_tensor("wqkl", [P, 4, 2, 512], f8e4, kind="ExternalInput").ap()
    wvh_d = nc.dram_tensor("wvh", [P, 4, 2, CD], f8e4, kind="ExternalInput").ap()
    wvl_d = nc.dram_tensor("wvl", [P, 4, 2, CD], f8e4, kind="ExternalInput").ap()
    wp_d = nc.dram_tensor("wp", [P, 2, D], bf16, kind="ExternalInput").ap()
    bqk_d = nc.dram_tensor("bqk", [P, 4], f32, kind="ExternalInput").ap()
    mask_d = nc.dram_tensor("mask", [P, P], f8e4, kind="ExternalInput").ap()
    y_d = nc.dram_tensor("y", [T, D], f32, kind="ExternalOutput").ap()

    with tile.TileContext(nc) as tc, contextlib.ExitStack() as ctx:
        const_p = ctx.enter_context(tc.tile_pool(name="const", bufs=1))
        w_p = ctx.enter_context(tc.tile_pool(name="weights", bufs=1))
        x_p = ctx.enter_context(tc.tile_pool(name="xin", bufs=1))
        qkt_p = ctx.enter_context(tc.tile_pool(name="qkt", bufs=1))
        v_p = ctx.enter_context(tc.tile_pool(name="vbuf", bufs=1))
        vt_p = ctx.enter_context(tc.tile_pool(name="vtmp", bufs=3))
        e_p = ctx.enter_context(tc.tile_pool(name="epool", bufs=8))
        attn_p = ctx.enter_context(tc.tile_pool(name="attn", bufs=1))
        small_p = ctx.enter_context(tc.tile_pool(name="small", bufs=8))
        y_p = ctx.enter_context(tc.tile_pool(name="ysb", bufs=3))
        # PSUM budget (8 banks of 2KB):
        #   sp   [128,1024] f32 x2   scores                     4 banks
        #   wps  [128, 512] f32 x2   qkv psum + proj psum       2 banks
        #   acc0/acc1 [128,4,65] f32 AV accumulators (2 heads)  2 banks
        psp = ctx.enter_context(tc.tile_pool(name="psp", bufs=2, space="PSUM"))

        # ---- constant / weight loads (startup: scalar queue is still free
        # of activations, so spread across both HWDGE queues) ----
        mask = const_p.tile([P, P], f8e4, name="mask_sb")
        nc.scalar.dma_start(mask, mask_d)
        maskb = mask.rearrange("p (o n) -> p o n", o=1).broadcast_to([P, 2, P])
        bqk = const_p.tile([P, 4], f32, name="bqk_sb")
        nc.scalar.dma_start(bqk, bqk_d)
        expb = const_p.tile([P, 1], f32, name="expb")
        nc.gpsimd.memset(expb, EXP_BIAS)

        wqkh = w_p.tile([P, 4, 2, 512], f8e4, name="wqkh")
        nc.sync.dma_start(wqkh, wqkh_d)
        wvh = w_p.tile([P, 4, 2, CD], f8e4, name="wvh")
        nc.scalar.dma_start(wvh, wvh_d)
        wvl = w_p.tile([P, 4, 2, CD], f8e4, name="wvl")
        nc.scalar.dma_start(wvl, wvl_d)
        wqkl = None
        if qk_split >= 2:
            wqkl = w_p.tile([P, 4, 2, 512], f8e4, name="wqkl")
            nc.scalar.dma_start(wqkl, wqkl_d)

        # x^T tiles: [128, blk, c, j, 512]; one DMA per 512-t block.
        xh_sb = x_p.tile([P, TB, 4, 2, 512], f8e4, name="xh")
        xl_sb = x_p.tile([P, TB, 4, 2, 512], f8e4, name="xl")

        def load_x_block(j, eng_h, eng_l):
            eng_h.dma_start(xh_sb[:, j], xh_d[:, j])
            eng_l.dma_start(xl_sb[:, j], xl_d[:, j])

        load_x_block(0, nc.sync, nc.scalar)

        wp_sb = w_p.tile([P, 2, D], bf16, name="wp")
        nc.scalar.dma_start(wp_sb, wp_d)

        # persistent activations
        qkt_sb = {(m, j): qkt_p.tile([P, 512], f8e3, name=f"qkt{m}_{j}",
                                     tag=f"qkt{m}_{j}")
                  for m in range(4) for j in range(TB)}
        # v tiles: [s-part, 2 (hi,lo), 4 heads x 65]; col 64 of each head is
        # the row-sum column (ALPHA in hi, 0 in lo).
        v_sb = [v_p.tile([P, 2, 4 * 65], f8e4, name=f"v{i}", tag=f"v{i}")
                for i in range(TT)]
        for i in range(TT):
            vg = v_sb[i].rearrange("p j (h c) -> p j h c", h=4)
            nc.gpsimd.memset(vg[:, 0, :, 64:65], ALPHA)
            nc.gpsimd.memset(vg[:, 1, :, 64:65], 0.0)
        attn_sb = [attn_p.tile([P, CD], bf16, name=f"attn{jt}", tag=f"attn{jt}")
                   for jt in range(TT)]
        attnT_sb = [attn_p.tile([P, T], bf16, name=f"attnT{g}", tag=f"attnT{g}")
                    for g in range(2)]

        def bcast2(ap_2d):
            """[128, n] -> [128, 2, n] with stride-0 middle dim."""
            n = ap_2d.shape[-1]
            return ap_2d.rearrange("p (o n) -> p o n", o=1).broadcast_to([P, 2, n])

        def qk_chunk(j, m, u):
            """q^T/k^T (DR fp8) for m-slice m, 256-col half u of t-block j."""
            ps = psp.tile([P, 256], f32, name="qkp", tag="wps")
            chains = [(wqkh, xh_sb)]
            if qk_split >= 2:
                chains.append((wqkl, xh_sb))
            if qk_split >= 3:
                chains.append((wqkh, xl_sb))
            n_ch = len(chains)
            for ci, (w4, x5) in enumerate(chains):
                for c in range(4):
                    nc.tensor.matmul(
                        ps,
                        w4[:, c, :, P * m : P * (m + 1)],
                        x5[:, j, c, :, 256 * u : 256 * (u + 1)],
                        start=(ci == 0 and c == 0),
                        stop=(ci == n_ch - 1 and c == 3),
                        perf_mode=DR,
                    )
            nc.vector.tensor_scalar(
                qkt_sb[(m, j)][:, 256 * u : 256 * (u + 1)], ps,
                QKT_MULT, bqk[:, m : m + 1],
                op0=mybir.AluOpType.mult, op1=mybir.AluOpType.add,
            )

        def v_chunk(j, ti):
            """V (DR fp8 hi+lo split) for s-tile 4j+ti."""
            g = 4 * j + ti
            ps = psp.tile([P, CD], f32, name="vp", tag="wps")
            xsl = slice(P * ti, P * (ti + 1))
            for ci, w4 in enumerate((wvh, wvl)):
                for c in range(4):
                    nc.tensor.matmul(
                        ps, xh_sb[:, j, c, :, xsl], w4[:, c],
                        start=(ci == 0 and c == 0), stop=False, perf_mode=DR,
                    )
            for c in range(4):
                nc.tensor.matmul(
                    ps, xl_sb[:, j, c, :, xsl], wvh[:, c],
                    start=False, stop=(c == 3), perf_mode=DR,
                )
            vtmp = vt_p.tile([P, CD], f32, name="vtmp", tag="vtmp")
            nc.vector.tensor_scalar_mul(vtmp, ps, V_MULT)
            vg = v_sb[g].rearrange("p j (h c) -> p j h c", h=4)
            vtg = vtmp.rearrange("p (h c) -> p h c", h=4)
            nc.gpsimd.tensor_copy(vg[:, 0, :, 0:64], vtg)
            nc.gpsimd.tensor_tensor(
                vg[:, 1, :, 0:64], vtg, vg[:, 0, :, 0:64],
                op=mybir.AluOpType.subtract,
            )

        def phase1_chunks(j):
            return ([(lambda m=m, u=u: qk_chunk(j, m, u))
                     for m in (0, 2, 1, 3) for u in range(2)]
                    + [(lambda ti=ti: v_chunk(j, ti)) for ti in range(4)])

        def proj_chunk(jt):
            """y[t-tile jt] = attn @ wp in bf16 (both 512-col halves, one
            merged output DMA on the SWDGE path)."""
            ysb = y_p.tile([P, D], f32, name="ysb", tag="ysb")
            for n in range(2):
                ps = psp.tile([P, 512], f32, name="yp", tag="wps")
                for g in range(2):
                    nc.tensor.matmul(
                        ps,
                        attnT_sb[g][:, P * jt : P * (jt + 1)],
                        wp_sb[:, g, 512 * n : 512 * (n + 1)],
                        start=(g == 0),
                        stop=(g == 1),
                    )
                nc.vector.tensor_copy(ysb[:, 512 * n : 512 * (n + 1)], ps)
            nc.gpsimd.dma_start(y_d[P * jt : P * (jt + 1), :], ysb)

        def attention(tb, fillers):
            """scores (e3m4) -> exp (e4m3) -> AV (DR, hi+lo via stride-0
            broadcast of e), both head pairs fused into one front/back
            software pipeline; AV trails by 2 s-tiles so the in-order PE
            queue never parks on an exp; one filler chunk is emitted per
            front to keep the PE fed (and at full p-state) while ACT works."""
            n_s = 4 * tb + 4
            LAG = 2
            state = {}

            def front(hp, i):
                mq, mk = hp, 2 + hp
                first = max(0, i - 4 * tb)
                c0 = P * first
                sps = psp.tile([P, 1024], f32, name="sp", tag="sp", bufs=2)
                for hh, pb in ((0, 0), (1, 64)):
                    nc.tensor.matmul(
                        sps[:, 512 * hh + c0 : 512 * hh + 512],
                        qkt_sb[(mk, i // 4)][pb : pb + H,
                                             P * (i % 4) : P * (i % 4 + 1)],
                        qkt_sb[(mq, tb)][pb : pb + H, c0:512],
                        start=True,
                        stop=True,
                    )
                et = e_p.tile([P, 1024], f8e4, name="et", tag="et")
                if first:
                    nc.scalar.activation(
                        et.rearrange("p (g c) -> p g c", g=2)[:, :, c0:512],
                        sps.rearrange("p (g c) -> p g c", g=2)[:, :, c0:512],
                        Exp, scale=EXP_SCALE, bias=expb,
                    )
                else:
                    nc.scalar.activation(et, sps, Exp, scale=EXP_SCALE,
                                         bias=expb)
                dj = i - 4 * tb
                etd = None
                if 0 <= dj <= 3:
                    etd = e_p.tile([P, 2 * P], f8e4, name="etd", tag="etd",
                                   bufs=4)
                    nc.gpsimd.tensor_tensor(
                        etd.rearrange("p (g c) -> p g c", g=2),
                        et.rearrange("p (g c) -> p g c", g=2)[
                            :, :, P * dj : P * (dj + 1)],
                        maskb,
                        op=mybir.AluOpType.mult,
                    )
                state[(hp, i)] = (et, etd)

            def back(hp, i):
                if i == 0:
                    state[("acc", hp)] = [
                        psp.tile([P, 4, 65], f32, name=f"acc{hh}",
                                 tag=f"acc{hh}", bufs=1) for hh in range(2)]
                acc = state[("acc", hp)]
                first = max(0, i - 4 * tb)
                dj = i - 4 * tb
                et, etd = state.pop((hp, i))
                for jj in range(first, 4):
                    for hh in range(2):
                        if jj == dj:
                            lhs_e = bcast2(etd[:, P * hh : P * (hh + 1)])
                        else:
                            lhs_e = bcast2(
                                et[:, 512 * hh + P * jj : 512 * hh + P * (jj + 1)]
                            )
                        h = 2 * hp + hh
                        nc.tensor.matmul(
                            acc[hh][:, jj, :],
                            lhs_e,
                            v_sb[i][:, :, 65 * h : 65 * (h + 1)],
                            start=(i == 0 and jj == 0),
                            stop=(i == 4 * tb + jj),
                            perf_mode=DR,
                            skip_group_check=True,
                        )
                if 0 <= dj <= 3:
                    jt = 4 * tb + dj
                    for hh in range(2):
                        h = 2 * hp + hh
                        rec = small_p.tile([P, 1], f32, name="rec", tag="rec")
                        nc.vector.reciprocal(rec, acc[hh][:, dj, 64:65])
                        nc.vector.tensor_scalar_mul(
                            attn_sb[jt][:, H * h : H * (h + 1)],
                            acc[hh][:, dj, 0:64],
                            rec,
                        )
                if i == n_s - 1:
                    # attn columns for this head pair are final:
                    # DMA-transpose them into attnT (XBAR) per 128x128 block.
                    for dj2 in range(4):
                        jt = 4 * tb + dj2
                        nc.sync.dma_start(
                            attnT_sb[hp][:, P * jt : P * (jt + 1)],
                            attn_sb[jt][:, P * hp : P * (hp + 1)],
                            transpose=True,
                        )

            stream = [(hp, i) for hp in range(2) for i in range(n_s)]
            for k, (hp, i) in enumerate(stream):
                front(hp, i)
                if fillers:
                    fillers.pop(0)()
                if len(fillers) > 10 and k % 2 == 0:
                    fillers.pop(0)()
                if k >= LAG:
                    back(*stream[k - LAG])
            for k in range(2 * n_s - LAG, 2 * n_s):
                back(*stream[k])
            while fillers:
                fillers.pop(0)()

        for ch in phase1_chunks(0):
            ch()
        load_x_block(1, nc.sync, nc.gpsimd)
        attention(0, phase1_chunks(1))
        load_x_block(2, nc.sync, nc.gpsimd)
        attention(1, [(lambda jt=jt: proj_chunk(jt)) for jt in range(0, 4)]
                  + phase1_chunks(2))
        load_x_block(3, nc.sync, nc.gpsimd)
        attention(2, [(lambda jt=jt: proj_chunk(jt)) for jt in range(4, 8)]
                  + phase1_chunks(3))
        attention(3, [(lambda jt=jt: proj_chunk(jt)) for jt in range(8, 12)])
        for jt in range(12, 16):
            proj_chunk(jt)

    nc.compile()
    return nc


def _get_module(qk_split: int = 1):
    if qk_split not in _CACHE:
        _CACHE[qk_split] = _build_module(qk_split)
    return _CACHE[qk_split]


def _x_layout(a):
    """x^T [1024, 2048] -> [p][blk][c][j][t512] with d = 256c + 128j + p."""
    return np.ascontiguousarray(
        a.reshape(4, 2, P, TB, 512).transpose(2, 3, 0, 1, 4)
    )


def _w_layout(a):
    """w [1024, n] -> [p][c][j][n] with row d = 256c + 128j + p."""
    n = a.shape[1]
    return np.ascontiguousarray(a.reshape(4, 2, P, n).transpose(2, 0, 1, 3))


def kernel(x, w_attn, b_attn, w_proj, b_proj, qk_split: int = 1,
           trace: bool = False):
    import ml_dtypes
    from concourse.bass_utils import run_bass_kernel_spmd

    e4 = np.dtype(ml_dtypes.float8_e4m3)  # IEEE variant: max 240, like the HW
    bf = np.dtype(ml_dtypes.bfloat16)

    x = np.asarray(x, dtype=np.float32)
    w_attn = np.asarray(w_attn, dtype=np.float32)
    b_attn = np.asarray(b_attn, dtype=np.float32)
    w_proj = np.asarray(w_proj, dtype=np.float32)
    b_proj = np.asarray(b_proj, dtype=np.float32)

    nc = _get_module(qk_split)

    mask = np.triu(np.ones((P, P), dtype=np.float32)).astype(e4)

    x_prep = []
    for b in range(B):
        x16 = x[b].T * S_X                      # [1024, 2048]
        xh = x16.astype(e4)
        xl = (x16 - xh.astype(np.float32)).astype(e4)
        x_prep.append((_x_layout(xh), _x_layout(xl)))

    in_maps = []
    for core in range(N_CORES):
        b, g = core // 4, core % 4
        c0 = CD * g
        wq = w_attn[:, c0 : c0 + CD]
        wk = w_attn[:, D + c0 : D + c0 + CD]
        wv = w_attn[:, 2 * D + c0 : 2 * D + c0 + CD]
        wqk = np.concatenate([wq, wk], axis=1) * S_W
        wqkh = wqk.astype(e4)
        wqkl = (wqk - wqkh.astype(np.float32)).astype(e4)
        wv_s = wv * S_W
        wvh = wv_s.astype(e4)
        wvl = (wv_s - wvh.astype(np.float32)).astype(e4)
        bq = b_attn[c0 : c0 + CD]
        bk = b_attn[D + c0 : D + c0 + CD]
        xh, xl = x_prep[b]
        in_maps.append(
            {
                "xh": xh,
                "xl": xl,
                "wqkh": _w_layout(wqkh),
                "wqkl": _w_layout(wqkl),
                "wvh": _w_layout(wvh),
                "wvl": _w_layout(wvl),
                "wp": np.ascontiguousarray(
                    w_proj[c0 : c0 + CD, :].reshape(2, P, D).transpose(1, 0, 2)
                ).astype(bf),
                "bqk": (np.concatenate([bq, bk]) * S_Q).reshape(4, P).T.copy(),
                "mask": mask,
            }
        )

    res = run_bass_kernel_spmd(
        nc, in_maps, core_ids=list(range(N_CORES)), trace=trace
    )

    out = np.zeros((B, T, D), dtype=np.float32)
    for core in range(N_CORES):
        out[core // 4] += np.asarray(res.results[core]["y"], dtype=np.float32)
    out += (b_proj + b_attn[2 * D :] @ w_proj)[None, None, :]
    if trace:
        kernel.last_result = res
    return out
